# revision 20
# baseline (speedup 1.0000x reference)
"""Trainium2 Bass kernel for nn_AutoregressiveHead (L=32, D=1024, H=512,
B=8192, P=2), data-parallel over batch across 8 NeuronCores.

    base = einsum('bd,ldh->blh', x, Wx);  pc = einsum('blp,lph->blh', y[:,pid], Wp)
    out  = einsum('blh,lh->bl', relu(base+pc+b1), W2) + b2

Structure (per core, B_core=1024, 64 blocks of (lg in 8 head-groups x bt in
8 batch-tiles), G=4 heads per group, PSUM [128,512] per head):

  * |W2| is folded into the W1 columns (relu positive-homogeneity), columns
    permuted so W2>=0 columns come first; stage 2 is then just two ScalarE
    relu+accum ranges (pos/neg) per head -- no second matmul.
  * k-outer / g-inner matmul order: the 4 heads of a group share the same
    lhsT (x k-tile strip), so the PE weight load (~53ns, measured serial,
    not hidden) is amortized 4x.  Measured 229.6 ns/matmul vs 261.9 for
    the lhsT-alternating order.  Deleting the redundant InstLdweights
    entirely measured SLOWER (245 ns/mm) -- keep them.
  * k-tiles 0..6 (896 dims) run in bf16.  The LAST x k-tile (dims 896..1023)
    and the parent/bias contraction ride ONE fp8e4m3 DoubleRow matmul
    (two K=128 planes per instruction, 2x rate): plane0 = [x_t7 | Wx_t7],
    plane1 = [y-labels+ones | dense parent rows + b1 row].  8 PE steps per
    block instead of 9 (-11%), and 512 fewer PE instructions.
  * fp8 range: all W1 columns are pre-scaled by SW=2^10 (exact exponent
    shift in bf16/fp8) so fp8 plane weights sit in e4m3 normal range; the
    relu undoes it for free via the ACT scale input (relu(z*SW * 1/SW)).
  * Accuracy: 1/8 of the contraction in fp8 -> rel err 0.0138 measured on
    HW (gate 2e-2; full-fp8 measures 0.0405, bf16 0.0024).  y in {0,1} and
    the ones row are exact in fp8; Wp error lands on pc which is ~2% of z.

Single-execution timeline (what the harness NTFF window sees) additionally
tuned:
  * prologue: per-k interleaved xT/wx DMAs -> first matmul at ~2us (was ~8:
    it waited on one monolithic 1.75MB xT transfer);
  * per-bt epilogue emitted inside the last head-group's loop so it
    overlaps the next bt's matmuls instead of serializing at program end;
  * multi-wait matmuls hand their extra sem-wait to the preceding zero-wait
    InstLdweights (same engine, monotonic sems -> equivalent), dropping the
    split_multi_waits PE nops from 255 to 7;
  * 1787/2048 PE-clock sem updates pruned (only ACT's stop-matmul ticks and
    a few DMA WAR ticks are ever waited on), wait thresholds remapped --
    sequencer-side saving, deadlock-free in TimelineSim, bit-identical HW;
  * one of the four heads per block drains on DVE instead of ACT
    (scalar_tensor_tensor max0*[sign/SW] then tensor_reduce -> signed sum
    straight into the pos column), rebalancing ACT 314->236us and
    shortening the end-of-program ACT tail.  rel err 0.013851 on HW.

Measured (8 axon TRN2 cores, repeats-delta method): v1 baseline 599-675us,
this kernel 333-373us best-rounds (timeline sim: 523 -> 427us; PE busy 96%).
Harness baseline was 964452 ns.  Rel err 0.013825 on HW, stable across runs.
Rejected by measurement: DVE reductions/stt (~500ns/instr overhead + this
walrus crashes on tensor_tensor_reduce), ldweights elision (slower on HW),
G=8 half-bank psum splits (2x instr count), pc on DVE (3 ops/head-bt too
dear), full/half fp8 (accuracy gate), hi/lo fp8 split (DR is 2x not 4x ->
loses), 2nd DR pair (err 0.0248 > gate), residual-corrected fp8 configs
(all need >= 8 PE steps anyway).
"""

import numpy as np
import ml_dtypes

import bass_rust
import concourse.bass as bass
import concourse.tile as tile
from concourse import mybir
from concourse.vector_clock import ScopedClock

BF16 = ml_dtypes.bfloat16
F8 = ml_dtypes.float8_e4m3

N_CORES = 8
B, D, H, L = 8192, 1024, 512, 32
B_CORE = B // N_CORES
PPART = 128
KTB = 7                       # bf16 k-tiles (dims 0..895)
G = 4                         # heads per PSUM group
N_LG = L // G
SW = 1024.0                   # weight pre-scale (2^10, exact)


class SplitDrainTileContext(tile.TileContext):
    """This container's walrus rejects >1 sem waits on the tail Drain ("Too
    many sync wait commands").  Redistribute the global-clock waits onto
    single-wait nops preceding the drain."""

    def _drain_and_barrier(self, tick_clock, wait_clock):
        probe = self.nc.sync.nop(nofuse=True)
        wait_clock.add_sem_waits(
            probe.ins, ScopedClock({None: tick_clock.global_clock})
        )
        si = probe.ins.sync_info
        waits = list(si.on_wait) if si is not None and si.on_wait else []
        if len(waits) > 1:
            si.on_wait = waits[:1]
            for w in waits[1:]:
                n = self.nc.sync.nop(nofuse=True)
                n.ins.sync_info = bass_rust.SyncInfo(on_wait=[w], on_update=[])
        self.nc.sync.drain()
        self.nc.all_engine_barrier()
        assert self.sems is not None
        popped = self.nc._tile_sem_poison_stack.pop()
        assert popped is self._sem_poison
        self.nc.clear_and_free_semaphores(list(self.sems.allocated().values()))
        self.nc.all_engine_barrier()


def prune_matmul_updates(nc):
    """Strip sem updates from PE matmuls whose clock tick no instruction
    waits on (~87% of them -- only ACT's stop-matmuls and a few DMA WAR
    ticks are referenced), remapping every wait threshold on that semaphore.
    Pure sequencer-side saving; validated deadlock-free in TimelineSim and
    bit-identical on HW."""
    from collections import Counter
    upd_count = Counter()
    for f in nc.m.functions:
        for blk in f.blocks:
            for inst in blk.instructions:
                if type(inst).__name__ != "InstMatmult":
                    continue
                si = inst.sync_info
                for u in (si.on_update or []) if si is not None else []:
                    if u.update_mode == "sem-inc":
                        upd_count[u.id] += 1
    if not upd_count:
        return 0
    sem_id = upd_count.most_common(1)[0][0]

    updates, waits = [], []
    for f in nc.m.functions:
        for blk in f.blocks:
            for inst in blk.instructions:
                si = inst.sync_info
                if si is None:
                    continue
                for u in (si.on_update or []):
                    if u.id == sem_id:
                        updates.append((inst, u))
                for w in (si.on_wait or []):
                    if w.id == sem_id:
                        assert w.wait_mode in ("sem-ge-imm", "sem-ge"), w.wait_mode
                        waits.append(w)

    referenced = set(w.wait_value for w in waits)
    referenced.add(len(updates))
    keep = [False] * (len(updates) + 1)
    for v in referenced:
        if 1 <= v <= len(updates):
            keep[v] = True

    new_rank = [0] * (len(updates) + 2)
    rank = 0
    for t in range(1, len(updates) + 1):
        if keep[t]:
            rank += 1
        new_rank[t] = rank
    nxt = [0] * (len(updates) + 2)
    r = rank
    for t in range(len(updates), 0, -1):
        if keep[t]:
            r = new_rank[t]
        nxt[t] = r
    n_del = 0
    for t, (inst, u) in enumerate(updates, start=1):
        if not keep[t]:
            si = inst.sync_info
            rest = [x for x in (si.on_update or []) if x is not u]
            inst.sync_info = bass_rust.SyncInfo(
                on_wait=list(si.on_wait) if si.on_wait else [], on_update=rest)
            n_del += 1
    for w in waits:
        v = w.wait_value
        if 1 <= v <= len(updates):
            w.wait_value = nxt[v] if nxt[v] > 0 else rank
        elif v > len(updates):
            w.wait_value = rank
    return n_del


def move_extra_waits_to_ldweights(nc):
    """An instruction may carry only 1 sem wait on this walrus.  For a PE
    matmul with >1 waits, move the extras onto the immediately preceding
    zero-wait InstLdweights (same engine, monotonic sems: waiting earlier
    is equivalent) so split_multi_waits doesn't have to inject a nop."""
    for f in nc.m.functions:
        for blk in f.blocks:
            prev_ldw = None
            for inst in blk.instructions:
                if inst.engine != mybir.EngineType.PE:
                    continue
                tn = type(inst).__name__
                if tn == "InstLdweights":
                    prev_ldw = inst
                    continue
                si = inst.sync_info
                waits = list(si.on_wait) if si is not None and si.on_wait else []
                if tn == "InstMatmult" and len(waits) > 1 and prev_ldw is not None:
                    lsi = prev_ldw.sync_info
                    lwaits = list(lsi.on_wait) if lsi is not None and lsi.on_wait else []
                    room = 1 - len(lwaits)
                    if room > 0:
                        moved = waits[:room]
                        lupd = list(lsi.on_update) if lsi is not None and lsi.on_update else []
                        prev_ldw.sync_info = bass_rust.SyncInfo(
                            on_wait=lwaits + moved, on_update=lupd)
                        upd = list(si.on_update) if si is not None and si.on_update else []
                        inst.sync_info = bass_rust.SyncInfo(
                            on_wait=waits[room:], on_update=upd)
                prev_ldw = None


def split_multi_waits(nc, max_waits: int = 1):
    """Hoist extra sem-waits onto single-wait NoOps inserted just before the
    instruction on the same engine (sems are monotonic, so waiting earlier
    on the same engine is equivalent)."""
    uid = 0
    for f in nc.m.functions:
        for blk in f.blocks:
            insts = blk.instructions
            new = []
            for inst in insts:
                si = inst.sync_info
                waits = list(si.on_wait) if si is not None and si.on_wait else []
                if len(waits) > max_waits:
                    for w in waits[:-max_waits]:
                        nop = mybir.InstNoOp(
                            name=f"splitw-{uid}", engine=inst.engine,
                            ins=[], outs=[],
                        )
                        uid += 1
                        nop.sync_info = bass_rust.SyncInfo(
                            on_wait=[w], on_update=[]
                        )
                        nc.register_instruction(nop, overwrite=True)
                        new.append(nop)
                    si.on_wait = waits[-max_waits:]
                new.append(inst)
            insts[:] = new


def build_program(n_bt: int, k_pos, repeats: int = 1):
    f32 = mybir.dt.float32
    bf16 = mybir.dt.bfloat16
    f8 = mybir.dt.float8e4
    bc = n_bt * PPART

    nc = bass.Bass("TRN2", target_bir_lowering=False, debug=False,
                   num_devices=N_CORES)

    xT_d = nc.dram_tensor("xT", [KTB * PPART, bc], bf16, kind="ExternalInput")
    xdr_d = nc.dram_tensor("xdr", [PPART, 2, bc], f8, kind="ExternalInput")
    wxp_d = nc.dram_tensor("wxp", [N_LG, KTB, G, PPART, H], bf16,
                           kind="ExternalInput")
    wdr_d = nc.dram_tensor("wdr", [N_LG, PPART, 2, G, H], f8,
                           kind="ExternalInput")
    b2r_d = nc.dram_tensor("b2r", [PPART, L], f32, kind="ExternalInput")
    # per-(g==3)-head signed unscale vectors: sign(W2)/SW, bf16-exact
    sgnb_d = nc.dram_tensor("sgnb", [PPART, N_LG, H], bf16, kind="ExternalInput")
    out_d = nc.dram_tensor("out", [bc, L], f32, kind="ExternalOutput")

    with SplitDrainTileContext(nc) as tc:
        with (
            tc.tile_pool(name="const", bufs=1) as const_pool,
            tc.tile_pool(name="wx", bufs=3) as wx_pool,
            tc.tile_pool(name="psum", bufs=8, space="PSUM") as psum_pool,
            tc.tile_pool(name="scratch", bufs=4) as scratch_pool,
            tc.tile_pool(name="outp", bufs=4) as out_pool,
        ):
            # Prologue DMA order matters: the first matmul needs only
            # xt[k=0] + wx[lg=0,k=0], so interleave per-k strips instead of
            # one monolithic xT transfer (first mm starts ~2us, not ~8us).
            xt_sb = const_pool.tile([PPART, KTB, bc], bf16, tag="xt")
            wx0_sb = wx_pool.tile([PPART, KTB, G, H], bf16, tag="wx")
            for k in range(KTB):
                nc.sync.dma_start(
                    xt_sb[:, k, :], xT_d.ap()[k * PPART:(k + 1) * PPART, :]
                )
                nc.sync.dma_start(
                    wx0_sb[:, k, :, :],
                    wxp_d.ap()[0, k].rearrange("g p h -> p g h"),
                )
            xdr_sb = const_pool.tile([PPART, 2, bc], f8, tag="xdr")
            nc.sync.dma_start(xdr_sb[:], xdr_d.ap())
            # prologue DMA in need-order: DR weights (~7us), sign vectors for
            # the first DVE drain (~11us), b2 (not read until the last lg)
            wdr0_sb = wx_pool.tile([PPART, 2, G, H], f8, tag="wdr")
            nc.sync.dma_start(wdr0_sb[:], wdr_d.ap()[0])
            sgn_sb = const_pool.tile([PPART, N_LG, H], bf16, tag="sgnb")
            nc.sync.dma_start(sgn_sb[:], sgnb_d.ap())
            b2_sb = const_pool.tile([PPART, L], f32, tag="b2")
            nc.sync.dma_start(b2_sb[:], b2r_d.ap())

            pos_sb = const_pool.tile([PPART, n_bt * L], f32, tag="pos")
            neg_sb = const_pool.tile([PPART, n_bt * L], f32, tag="neg")
            nc.vector.memset(pos_sb[:], 0.0)
            nc.vector.memset(neg_sb[:], 0.0)

            inv_sw = 1.0 / SW
            first = True
            for _rep in range(repeats):
                for lg in range(N_LG):
                    if first:
                        wx_sb, wdr_sb = wx0_sb, wdr0_sb
                        first = False
                    else:
                        wx_sb = wx_pool.tile([PPART, KTB, G, H], bf16, tag="wx")
                        for k in range(KTB):
                            nc.sync.dma_start(
                                wx_sb[:, k, :, :],
                                wxp_d.ap()[lg, k].rearrange("g p h -> p g h"),
                            )
                        wdr_sb = wx_pool.tile([PPART, 2, G, H], f8, tag="wdr")
                        nc.sync.dma_start(wdr_sb[:], wdr_d.ap()[lg])
                    for bt in range(n_bt):
                        bsl = slice(bt * PPART, (bt + 1) * PPART)
                        ps = [
                            psum_pool.tile([PPART, H], f32, tag="ps", name="ps")
                            for _ in range(G)
                        ]
                        for k in range(KTB):
                            for g in range(G):
                                nc.tensor.matmul(
                                    ps[g][:], lhsT=xt_sb[:, k, bsl],
                                    rhs=wx_sb[:, k, g, :],
                                    start=(k == 0), stop=False,
                                )
                        for g in range(G):
                            nc.tensor.matmul(
                                ps[g][:], lhsT=xdr_sb[:, :, bsl],
                                rhs=wdr_sb[:, :, g, :],
                                start=False, stop=True,
                                perf_mode=mybir.MatmulPerfMode.DoubleRow,
                            )
                        for g in range(G):
                            head = lg * G + g
                            kl = int(k_pos[head])
                            col = bt * L + head
                            sc = scratch_pool.tile([PPART, H], bf16, tag="sc")
                            if g == G - 1:
                                # offload this head's stage-2 to the idle
                                # DVE: signed relu (max(z,0) * sign/SW) then
                                # add-reduce -> pos col (neg stays 0)
                                nc.vector.scalar_tensor_tensor(
                                    sc[:], ps[g][:], 0.0, sgn_sb[:, lg, :],
                                    mybir.AluOpType.max, mybir.AluOpType.mult,
                                )
                                nc.vector.tensor_reduce(
                                    pos_sb[:, col:col + 1], sc[:],
                                    mybir.AxisListType.X, mybir.AluOpType.add,
                                )
                                continue
                            if kl > 0:
                                nc.scalar.activation(
                                    sc[:, :kl], ps[g][:, :kl],
                                    mybir.ActivationFunctionType.Relu,
                                    scale=inv_sw,
                                    accum_out=pos_sb[:, col:col + 1],
                                )
                            if kl < H:
                                nc.scalar.activation(
                                    sc[:, kl:], ps[g][:, kl:],
                                    mybir.ActivationFunctionType.Relu,
                                    scale=inv_sw,
                                    accum_out=neg_sb[:, col:col + 1],
                                )
                        if lg == N_LG - 1:
                            # bt's pos/neg cols are complete after the last
                            # head group -- emit its epilogue here so it
                            # overlaps the next bt's matmuls instead of
                            # serializing at program end.
                            o = out_pool.tile([PPART, L], f32, tag="o")
                            nc.vector.tensor_tensor(
                                o[:], pos_sb[:, bt * L:(bt + 1) * L],
                                neg_sb[:, bt * L:(bt + 1) * L],
                                mybir.AluOpType.subtract,
                            )
                            nc.vector.tensor_tensor(
                                o[:], o[:], b2_sb[:], mybir.AluOpType.add,
                            )
                            nc.sync.dma_start(
                                out_d.ap()[bt * PPART:(bt + 1) * PPART, :], o[:]
                            )

    prune_matmul_updates(nc)
    move_extra_waits_to_ldweights(nc)
    split_multi_waits(nc)
    return nc


def prep_host(x, y, Wx, Wp, b1, W2, b2, parent_idx, n_bt: int = 8):
    x = np.asarray(x, np.float32)
    y = np.asarray(y, np.float32)
    Wx = np.asarray(Wx, np.float32)
    Wp = np.asarray(Wp, np.float32)
    b1 = np.asarray(b1, np.float32)
    W2 = np.asarray(W2, np.float32)
    b2 = np.asarray(b2, np.float32)
    parent_idx = np.asarray(parent_idx)
    NP = parent_idx.shape[1]
    bc = n_bt * PPART

    # |W2| fold + sign-partition permutation of the H axis (per head)
    s = np.abs(W2)
    k_pos = np.zeros(L, np.int64)
    perm = np.zeros((L, H), np.int64)
    for l in range(L):
        posm = W2[l] >= 0
        perm[l] = np.concatenate([np.where(posm)[0], np.where(~posm)[0]])
        k_pos[l] = int(posm.sum())

    DB = KTB * PPART
    wxp = np.empty((N_LG, KTB, G, PPART, H), BF16)
    wdr = np.zeros((N_LG, PPART, 2, G, H), F8)
    for l in range(L):
        m = (Wx[l] * s[l][None, :])[:, perm[l]] * SW
        lg, g = l // G, l % G
        wxp[lg, :, g] = m[:DB].reshape(KTB, PPART, H).astype(BF16)
        wdr[lg, :, 0, g, :] = m[DB:].astype(F8)
        wpa = np.zeros((PPART, H), np.float32)
        for j in range(NP):
            wpa[parent_idx[l, j]] += Wp[l, j]
        wpa[L] = b1[l]
        wdr[lg, :, 1, g, :] = ((wpa * s[l][None, :])[:, perm[l]] * SW).astype(F8)

    b2r = np.broadcast_to(b2[None, :], (PPART, L)).astype(np.float32).copy()

    # signed unscale rows for the DVE-offloaded heads (g == G-1), permuted
    # order = [+1/SW x kl, -1/SW x (H-kl)]
    sgnb = np.empty((PPART, N_LG, H), BF16)
    for lg in range(N_LG):
        l = lg * G + (G - 1)
        row = np.full(H, -1.0 / SW, np.float32)
        row[:int(k_pos[l])] = 1.0 / SW
        sgnb[:, lg, :] = row.astype(BF16)[None, :]

    in_maps = []
    for c in range(N_CORES):
        xs = x[c * bc:(c + 1) * bc]
        ys = y[c * bc:(c + 1) * bc]
        xT = np.ascontiguousarray(xs[:, :DB].T).astype(BF16)
        xdr = np.zeros((PPART, 2, bc), F8)
        xdr[:, 0, :] = np.ascontiguousarray(xs[:, DB:].T).astype(F8)
        ya = np.zeros((PPART, bc), np.float32)
        ya[:L] = ys.T
        ya[L] = 1.0
        xdr[:, 1, :] = ya.astype(F8)
        in_maps.append({"xT": xT, "xdr": xdr, "wxp": wxp, "wdr": wdr,
                        "b2r": b2r, "sgnb": sgnb})
    return in_maps, k_pos


_CACHE = {}


def kernel(x, y, Wx, Wp, b1, W2, b2, parent_idx):
    from concourse.bass_utils import run_bass_kernel_spmd

    x = np.asarray(x)
    n_bt = x.shape[0] // N_CORES // PPART
    in_maps, k_pos = prep_host(x, y, Wx, Wp, b1, W2, b2, parent_idx, n_bt=n_bt)

    key = (n_bt, tuple(int(v) for v in k_pos))
    if key not in _CACHE:
        _CACHE[key] = build_program(n_bt, k_pos)
    nc = _CACHE[key]

    res = run_bass_kernel_spmd(nc, in_maps, core_ids=list(range(N_CORES)))
    out = np.concatenate([res.results[c]["out"] for c in range(N_CORES)], axis=0)
    return out.astype(np.float32)
